# revision 26
# baseline (speedup 1.0000x reference)
"""Trainium2 Bass kernel for the CA2 dense-transformer problem — fp8 version.

Math (per batch b of 8, S=2048, D=512):
    Q1 = X @ W_xq.T + b_xq            # [S, D]
    Q2 = Y @ W_yq.T + b_yq
    Qc = concat(Q1, Q2, -1)           # [S, 2D]
    K  = Qc @ W_fk.T + b_fk
    V  = Qc @ W_fv.T + b_fv
    out = X + Y + softmax(Q1 K^T / sqrt(D)) V + softmax(Q2 K^T / sqrt(D)) V

Implementation notes:
  * All matmuls run in fp8e4 (e4m3) with MatmulPerfMode.DoubleRow: each
    instruction contracts 2x128 with stationary [128,2,<=128] and moving
    [128,2,<=256], accumulating fp32 in PSUM.  ~2.4x the fp32r row rate.
  * PSUM accumulation groups are 2KB-region scoped: exactly one start=True
    (zeroes the region) and one stop=True per region per group; multiple
    matmuls may hit different subranges of the region inside one group, but
    a second start while a group is live corrupts the region.
  * Scores are computed transposed [k, q] (keys on partitions) so that
    exp(scores) chunks feed P @ V directly as DoubleRow stationary operands.
    Softmax denominators come from extra tiny PE matmuls (moving = fp8 ones
    column) accumulated in a dedicated PSUM bank; softmax scale 1/sqrt(D) is
    folded into the exp activation's scale input.
  * Engine placement: exp on Act (64 x [128,1024] calls); psum->fp8 drains
    split so both copy engines run concurrently (X-side Q^T + K^T with bias
    on Act via Identity-activation, Y-side Q^T + V on DVE); residual init
    X+Y on GPSIMD (SBUF-only — GPSIMD cannot access PSUM); reciprocal +
    final rescale-accumulate on DVE; all matmuls on PE.  Inputs stream on
    two DMA queues (weights on sync, xt/yt/x/y on the Pool queue); outputs
    stream per-tile from the pass-2 loop.
  * The attention inner loop is software-pipelined: scores/exp for item i+1
    are emitted before PV/den for item i, keeping Act busy across q-block
    boundaries.
  * Residual X + Y rides in bf16 (host-converted), attention I/O in fp8;
    output fp32.  Measured end-to-end rel err vs f32 reference ~2e-3.
"""

import sys

if "/opt/trn_rl_repo" not in sys.path:
    sys.path.insert(0, "/opt/trn_rl_repo")

import numpy as np

import concourse.bass as bass  # noqa: F401
import concourse.mybir as mybir
import concourse.tile as tile
from concourse import bacc
from concourse.bass_utils import run_bass_kernel_spmd

P = 128           # SBUF partitions
S = 2048          # tokens per batch
D = 512           # feature dim
NQT = S // P      # 16 token tiles
NET = D // P      # 4 feature tiles of D
NCT = 2 * D // P  # 8 feature tiles of 2D
NSS = S // 512    # 4 512-wide token column slices
QB = 256          # queries per attention block
NQB = S // QB     # 8
FP = mybir.dt.float32
F8 = mybir.dt.float8e4
BF = mybir.dt.bfloat16
DR = mybir.MatmulPerfMode.DoubleRow
SCALE = float(1.0 / np.sqrt(np.float32(D)))

_CACHE = {}


def _build(reps: int = 1):
    nc = bacc.Bacc("TRN2", target_bir_lowering=False, debug=False)

    xt_d = nc.dram_tensor("xt", [NET, P, S], F8, kind="ExternalInput")
    yt_d = nc.dram_tensor("yt", [NET, P, S], F8, kind="ExternalInput")
    x_d = nc.dram_tensor("x", [NQT, P, D], BF, kind="ExternalInput")
    y_d = nc.dram_tensor("y", [NQT, P, D], BF, kind="ExternalInput")
    wxq_d = nc.dram_tensor("wxq", [NET, P, D], F8, kind="ExternalInput")
    wyq_d = nc.dram_tensor("wyq", [NET, P, D], F8, kind="ExternalInput")
    wfk_d = nc.dram_tensor("wfk", [NCT, P, D], F8, kind="ExternalInput")
    wfv_d = nc.dram_tensor("wfv", [NCT, P, D], F8, kind="ExternalInput")
    bq_d = nc.dram_tensor("bq", [P, 12], FP, kind="ExternalInput")
    bfv_d = nc.dram_tensor("bfv", [P, D], FP, kind="ExternalInput")
    out_d = nc.dram_tensor("out", [NQT, P, D], FP, kind="ExternalOutput")

    Exp = mybir.ActivationFunctionType.Exp
    Ident = mybir.ActivationFunctionType.Identity
    mult = mybir.AluOpType.mult
    add = mybir.AluOpType.add

    with tile.TileContext(nc) as tc:
        for _rep in range(reps):
            with (
                tc.tile_pool(name="main", bufs=1) as main,
            ):
                q1t = main.tile([P, NET, S], F8, tag="q1t")
                q2t = main.tile([P, NET, S], F8, tag="q2t")
                kft = main.tile([P, NET, S], F8, tag="kft")
                vf = main.tile([P, NQT, D], F8, tag="vf")
                racc = main.tile([P, NQT, D], FP, tag="racc")
                bq = main.tile([P, 12], FP, tag="bq")
                ones8 = main.tile([P, 2, 1], F8, tag="ones8")
                ones_f = main.tile([P, 2], FP, tag="ones_f")
                nc.sync.dma_start(bq[:], bq_d[:])
                nc.vector.memset(ones_f[:], 1.0)
                nc.vector.tensor_copy(ones8[:, :, 0], ones_f[:])

                # ---- Stage A+B pipelined per 512-token slice:
                #      Q^T projections -> K^T / V for that slice ----
                with (
                    tc.tile_pool(name="esp", bufs=4) as esp,
                    tc.tile_pool(name="rcp", bufs=4) as rcp,
                    tc.tile_pool(name="pss", bufs=2, space="PSUM") as pss,
                ):
                  pre = []

                  def sc_exp(qsrc_p, qb, ktq):
                    ps = pss.tile([P, 4, 256], FP, tag="ps", name="ps")
                    for reg in range(2):
                        k = 0
                        for sl in range(2):
                            kt = 4 * ktq + 2 * reg + sl
                            for pr in range(2):
                                nc.tensor.matmul(
                                    ps[:, 2 * reg + sl],
                                    kft[:, 2 * pr : 2 * pr + 2,
                                        kt * P : (kt + 1) * P],
                                    qsrc_p[:, 2 * pr : 2 * pr + 2,
                                           qb * QB : (qb + 1) * QB],
                                    start=k == 0,
                                    stop=k == 3,
                                    perf_mode=DR,
                                )
                                k += 1
                    es = esp.tile([P, 4, 256], F8, tag="es", name="es")
                    nc.scalar.activation(es[:], ps[:], Exp, scale=SCALE)
                    return (es, qsrc_p, qb, ktq)

                  with (
                    tc.tile_pool(name="stAB", bufs=1) as stAB,
                    tc.tile_pool(name="psAB", bufs=2, space="PSUM") as psAB,
                ):
                    xt = stAB.tile([P, NET, S], F8, tag="xt")
                    yt = stAB.tile([P, NET, S], F8, tag="yt")
                    wxq = stAB.tile([P, NET, D], F8, tag="wxq")
                    wyq = stAB.tile([P, NET, D], F8, tag="wyq")
                    wfv = stAB.tile([P, NCT, D], F8, tag="wfv")
                    wfk = stAB.tile([P, NCT, D], F8, tag="wfk")
                    bfv = stAB.tile([P, 2, D], FP, tag="bfv")
                    # weights on the sync queue; xt/yt token slices + x/y
                    # residuals on the Pool queue — two parallel DMA streams,
                    # so the first A-group is gated at ~1us.
                    for dt in range(NET):
                        nc.sync.dma_start(wxq[:, dt], wxq_d[dt])
                        nc.sync.dma_start(wyq[:, dt], wyq_d[dt])
                    for ct in range(NCT):
                        nc.sync.dma_start(wfk[:, ct], wfk_d[ct])
                        nc.sync.dma_start(wfv[:, ct], wfv_d[ct])
                    for i in range(2):
                        nc.sync.dma_start(bfv[:, i], bfv_d[:])
                    for ssd in range(NSS):
                        for dt in range(NET):
                            nc.gpsimd.dma_start(
                                xt[:, dt, ssd * 512 : (ssd + 1) * 512],
                                xt_d[dt, :, ssd * 512 : (ssd + 1) * 512],
                            )
                        for dt in range(NET):
                            nc.gpsimd.dma_start(
                                yt[:, dt, ssd * 512 : (ssd + 1) * 512],
                                yt_d[dt, :, ssd * 512 : (ssd + 1) * 512],
                            )
                    # residual loads on the Pool queue too (needed late)
                    txy = stAB.tile([P, NQT, 2, D], BF, tag="txy")
                    for qt in range(NQT):
                        nc.gpsimd.dma_start(txy[:, qt, 0], x_d[qt])
                        nc.gpsimd.dma_start(txy[:, qt, 1], y_d[qt])

                    def a_q(src, w, qdst, bcol, ssp, et, drain):
                        # two 512-col psum groups (one per region), one
                        # 1024-wide drain with per-et bias on Act or DVE
                        ps = psAB.tile([P, 1024], FP, tag="psAB", name="psA")
                        for r in range(2):
                            ss = 2 * ssp + r
                            k = 0
                            for pr in range(2):
                                for h in range(2):
                                    nc.tensor.matmul(
                                        ps[:, r * 512 + h * 256
                                           : r * 512 + (h + 1) * 256],
                                        w[:, 2 * pr : 2 * pr + 2,
                                          et * P : (et + 1) * P],
                                        src[:, 2 * pr : 2 * pr + 2,
                                            ss * 512 + h * 256
                                            : ss * 512 + (h + 1) * 256],
                                        start=k == 0,
                                        stop=k == 3,
                                        perf_mode=DR,
                                    )
                                    k += 1
                        dst = qdst[:, et, ssp * 1024 : (ssp + 1) * 1024]
                        bias = bq[:, bcol + et : bcol + et + 1]
                        if drain == "act":
                            nc.scalar.activation(dst, ps[:], Ident, bias=bias)
                        else:
                            nc.vector.tensor_scalar_add(dst, ps[:], bias)

                    def b_v(ktp):
                        ps = psAB.tile([P, 2, D], FP, tag="psAB", name="psBv")
                        for r in range(2):
                            kt = 2 * ktp + r
                            k = 0
                            for pr in range(4):
                                qc = q1t if pr < 2 else q2t
                                d0 = (2 * pr) % NET
                                for h in range(2):
                                    nc.tensor.matmul(
                                        ps[:, r, h * 256 : (h + 1) * 256],
                                        qc[:, d0 : d0 + 2,
                                           kt * P : (kt + 1) * P],
                                        wfv[:, 2 * pr : 2 * pr + 2,
                                            h * 256 : (h + 1) * 256],
                                        start=k == 0,
                                        stop=k == 7,
                                        perf_mode=DR,
                                    )
                                    k += 1
                        nc.vector.tensor_add(
                            vf[:, 2 * ktp : 2 * ktp + 2, :], ps[:], bfv[:]
                        )

                    def b_k(ssp, et):
                        ps = psAB.tile([P, 1024], FP, tag="psAB", name="psBk")
                        for r in range(2):
                            ss = 2 * ssp + r
                            k = 0
                            for pr in range(4):
                                qc = q1t if pr < 2 else q2t
                                d0 = (2 * pr) % NET
                                for h in range(2):
                                    nc.tensor.matmul(
                                        ps[:, r * 512 + h * 256
                                           : r * 512 + (h + 1) * 256],
                                        wfk[:, 2 * pr : 2 * pr + 2,
                                            et * P : (et + 1) * P],
                                        qc[:, d0 : d0 + 2,
                                           ss * 512 + h * 256
                                           : ss * 512 + (h + 1) * 256],
                                        start=k == 0,
                                        stop=k == 7,
                                        perf_mode=DR,
                                    )
                                    k += 1
                        nc.scalar.activation(
                            kft[:, et, ssp * 1024 : (ssp + 1) * 1024],
                            ps[:],
                            Ident,
                            bias=bq[:, 8 + et : 9 + et],
                        )

                    # X-side Q + K drains on Act, Y-side Q + V drains on DVE,
                    # interleaved so both engines drain concurrently
                    # all A first (PE fills the A-drain wait with the
                    # second slice-pair), then all B
                    for ssp in range(2):
                        for et in range(NET):
                            a_q(xt, wxq, q1t, 0, ssp, et, "act")
                            a_q(yt, wyq, q2t, 4, ssp, et, "dve")
                    # all K before all V: kft gates the attention ramp
                    # (exp for qb0 sweeps every kt), V is only needed at PV
                    for ssp in range(2):
                        for et in range(NET):
                            b_k(ssp, et)
                    # prologue: first two score/exp items run while the PE
                    # finishes the V projections (PV for them is deferred)
                    pre.append(sc_exp(q1t, 0, 0))
                    pre.append(sc_exp(q1t, 0, 1))
                    for ktp in range(8):
                        b_v(ktp)

                    # residual init on Pool (SBUF only)
                    for qt in range(NQT):
                        nc.gpsimd.tensor_add(
                            racc[:, qt], txy[:, qt, 0], txy[:, qt, 1]
                        )

                  # ---- Attention: 2 passes x 8 q-blocks of 256 ----
                  if True:
                    with (
                        tc.tile_pool(name="pso", bufs=1, space="PSUM") as pso,
                        tc.tile_pool(name="psd", bufs=2, space="PSUM") as psd,
                    ):
                        # Software-pipelined: scores/exp for item i+1 are
                        # emitted before PV/den for item i, so the Act engine
                        # never waits on the PE's PV tail at qb boundaries.
                        state = {"po": None, "den": None}

                        def emit_pv(es, qsrc_p, qb, ktq):
                            if ktq == 0:
                                state["po"] = [
                                    pso.tile([P, D], FP, name=f"po{qs}",
                                             tag=f"po{qs}")
                                    for qs in range(2)
                                ]
                                state["den"] = psd.tile(
                                    [P, 2], FP, name="den", tag="den"
                                )
                            po, den = state["po"], state["den"]
                            for pp in range(2):
                                kt0 = 4 * ktq + 2 * pp
                                for qs in range(2):
                                    st = es[:, 2 * pp : 2 * pp + 2,
                                            qs * P : (qs + 1) * P]
                                    for h in range(2):
                                        nc.tensor.matmul(
                                            po[qs][:, h * 256 : (h + 1) * 256],
                                            st,
                                            vf[:, kt0 : kt0 + 2,
                                               h * 256 : (h + 1) * 256],
                                            start=(ktq == 0 and pp == 0
                                                   and h == 0),
                                            stop=(ktq == 3 and pp == 1
                                                  and h == 1),
                                            perf_mode=DR,
                                        )
                                    nc.tensor.matmul(
                                        den[:, qs : qs + 1],
                                        st,
                                        ones8[:],
                                        start=(ktq == 0 and pp == 0
                                               and qs == 0),
                                        stop=(ktq == 3 and pp == 1
                                              and qs == 1),
                                        perf_mode=DR,
                                    )
                            if ktq == 3:
                                rec = rcp.tile([P, 2], FP, tag="rec",
                                               name="rec")
                                nc.vector.reciprocal(rec[:], den[:])
                                for qs in range(2):
                                    qt_i = qb * 2 + qs
                                    nc.vector.scalar_tensor_tensor(
                                        racc[:, qt_i],
                                        po[qs][:],
                                        rec[:, qs : qs + 1],
                                        racc[:, qt_i],
                                        op0=mult,
                                        op1=add,
                                    )
                                    if qsrc_p is q2t:
                                        nc.sync.dma_start(
                                            out_d[qt_i], racc[:, qt_i]
                                        )

                        items = [
                            (qsrc, qb, ktq)
                            for qsrc in (q1t, q2t)
                            for qb in range(NQB)
                            for ktq in range(NQT // 4)
                        ]
                        for it in items[len(pre):]:
                            pre.append(sc_exp(*it))
                            emit_pv(*pre.pop(0))
                        for ent in pre:
                            emit_pv(*ent)

    nc.compile()
    return nc


def get_nc(reps: int = 1):
    if reps not in _CACHE:
        _CACHE[reps] = _build(reps)
    return _CACHE[reps]


def make_in_maps(X, Y, W_xq, b_xq, W_yq, b_yq, W_fk, b_fk, W_fv, b_fv):
    """Host-side layout prep (transpose/reshape/dtype-convert only) and
    per-core sharding over batch."""
    import ml_dtypes

    f32 = np.float32
    e4 = ml_dtypes.float8_e4m3
    bf = ml_dtypes.bfloat16

    def q8(a, shape):
        return np.ascontiguousarray(
            np.asarray(a, dtype=f32).reshape(shape)
        ).astype(e4)

    wxq = q8(W_xq.T, (NET, P, D))
    wyq = q8(W_yq.T, (NET, P, D))
    wfk = q8(W_fk.T, (NCT, P, D))
    wfv = q8(W_fv.T, (NCT, P, D))
    bq = np.empty((P, 12), f32)
    bq[:, 0:4] = np.asarray(b_xq, f32).reshape(NET, P).T
    bq[:, 4:8] = np.asarray(b_yq, f32).reshape(NET, P).T
    bq[:, 8:12] = np.asarray(b_fk, f32).reshape(NET, P).T
    bfv = np.ascontiguousarray(
        np.broadcast_to(np.asarray(b_fv, f32), (P, D))
    )

    in_maps = []
    for b in range(X.shape[0]):
        in_maps.append(
            {
                "xt": q8(X[b].T, (NET, P, S)),
                "yt": q8(Y[b].T, (NET, P, S)),
                "x": np.asarray(X[b], f32).reshape(NQT, P, D).astype(bf),
                "y": np.asarray(Y[b], f32).reshape(NQT, P, D).astype(bf),
                "wxq": wxq,
                "wyq": wyq,
                "wfk": wfk,
                "wfv": wfv,
                "bq": bq,
                "bfv": bfv,
            }
        )
    return in_maps


def kernel(X, Y, W_xq, b_xq, W_yq, b_yq, W_fk, b_fk, W_fv, b_fv):
    X = np.asarray(X, np.float32)
    Y = np.asarray(Y, np.float32)
    B = X.shape[0]
    nc = get_nc()
    in_maps = make_in_maps(
        X, Y,
        np.asarray(W_xq, np.float32), np.asarray(b_xq, np.float32),
        np.asarray(W_yq, np.float32), np.asarray(b_yq, np.float32),
        np.asarray(W_fk, np.float32), np.asarray(b_fk, np.float32),
        np.asarray(W_fv, np.float32), np.asarray(b_fv, np.float32),
    )
    res = run_bass_kernel_spmd(nc, in_maps, list(range(B)))
    out = np.stack([res.results[b]["out"].reshape(S, D) for b in range(B)])
    return out
